# revision 5
# baseline (speedup 1.0000x reference)
"""Trainium2 Bass kernel for nn_EquivariantConvolution (gnn_message_passing).

Math (per edge e):
    h  = relu(edge_feats @ W1 + b1)            [E,128]
    rw = (h @ W2 + b2) -> [E, 16, 48]
    fe = f[U]                                  [E,16,3]
    tmp[e,m,k] = sum_d fe[e,m,d] * basis[e,d,k]        (k = r*3+dd, 9)
    out[e,i,dd] = sum_{m,r} rw[e,i,m*3+r] * tmp[e,m,r*3+dd]

Sharding: edges split across 8 cores (40000 each, padded to 40960);
f + MLP weights replicated. Device layout: edge j of a 128-edge tile on
SBUF partition j%128; MLP on the tensor engine (float32r), f[U] via
dma_gather, per-edge contractions on the vector engine with broadcast APs.
"""
import sys

sys.path.insert(0, "/opt/trn_rl_repo")

import numpy as np
import concourse.bass as bass
import concourse.bacc as bacc
import concourse.mybir as mybir
import concourse.tile as tile
from concourse.bass_utils import run_bass_kernel_spmd
from contextlib import ExitStack

# problem constants (hardcoded per harness contract)
E = 320000
N = 10000
M1 = 16
M2 = 16
D1 = 3
D2 = 3
NREPS = 3
EDGE_DIM = 32
HIDDEN = 128
RW = NREPS * M1 * M2  # 768

NCORES = 8
ES = E // NCORES          # 40000 edges per core
ESP = 40960               # padded to 320 tiles of 128
NTILES = ESP // 128       # 320
BLK = 16                  # tiles per block
NBLK = NTILES // BLK      # 20
EBLK = BLK * 128          # 2048 edges per block
FPAD = 64                 # f rows padded to 64 floats (256B) for dma_gather

_CACHE = {}


def _build(reps=1):
    dt = mybir.dt
    nc = bacc.Bacc("TRN2", target_bir_lowering=False, debug=False,
                   num_devices=NCORES)

    # DRAM tensors (per-core shards fed via in_maps)
    efT_d = nc.dram_tensor("efT", [EDGE_DIM, ESP], dt.float32r, kind="ExternalInput").ap()
    basis_d = nc.dram_tensor("basisp", [128, NTILES * 27], dt.float32, kind="ExternalInput").ap()
    uw_d = nc.dram_tensor("uw", [128, ESP // 16], dt.int16, kind="ExternalInput").ap()
    f_d = nc.dram_tensor("fpad", [N, FPAD], dt.float32, kind="ExternalInput").ap()
    w1_d = nc.dram_tensor("w1", [EDGE_DIM, HIDDEN], dt.float32r, kind="ExternalInput").ap()
    b1_d = nc.dram_tensor("b1", [HIDDEN, 1], dt.float32, kind="ExternalInput").ap()
    w2_d = nc.dram_tensor("w2", [HIDDEN, RW], dt.float32r, kind="ExternalInput").ap()
    b2_d = nc.dram_tensor("b2", [1, RW], dt.float32r, kind="ExternalInput").ap()
    ones_d = nc.dram_tensor("ones1", [1, 128], dt.float32r, kind="ExternalInput").ap()
    out_d = nc.dram_tensor("outp", [128, NTILES * 48], dt.float32, kind="ExternalOutput").ap()

    with tile.TileContext(nc) as tc, ExitStack() as ctx:
        cpool = ctx.enter_context(tc.tile_pool(name="const", bufs=1))
        inpool = ctx.enter_context(tc.tile_pool(name="in", bufs=2))
        hpool = ctx.enter_context(tc.tile_pool(name="h", bufs=2))
        wpool = ctx.enter_context(tc.tile_pool(name="work", bufs=3))
        opool = ctx.enter_context(tc.tile_pool(name="out", bufs=2))
        pps = ctx.enter_context(tc.tile_pool(name="psA", bufs=2, space="PSUM"))
        ppr = ctx.enter_context(tc.tile_pool(name="psB", bufs=2, space="PSUM"))

        # constants
        w1_sb = cpool.tile([EDGE_DIM, HIDDEN], dt.float32r)
        nc.sync.dma_start(w1_sb[:], w1_d[:])
        b1_sb = cpool.tile([HIDDEN, 1], dt.float32)
        nc.sync.dma_start(b1_sb[:], b1_d[:])
        w2_sb = cpool.tile([HIDDEN, RW], dt.float32r)
        nc.sync.dma_start(w2_sb[:], w2_d[:])
        b2_sb = cpool.tile([1, RW], dt.float32r)
        nc.sync.dma_start(b2_sb[:], b2_d[:])
        ones_sb = cpool.tile([1, 128], dt.float32r)
        nc.sync.dma_start(ones_sb[:], ones_d[:])
        uw_sb = cpool.tile([128, ESP // 16], dt.int16)
        nc.sync.dma_start(uw_sb[:], uw_d[:])

        def body():
            _body(nc, tc, dt, cpool, inpool, hpool, wpool, opool, pps, ppr,
                  efT_d, basis_d, f_d, uw_sb, w1_sb, b1_sb, w2_sb, b2_sb,
                  ones_sb, out_d)

        if reps == 1:
            body()
        else:
            with tc.For_i(0, reps, 1):
                body()

    nc.compile()
    return nc


def _body(nc, tc, dt, cpool, inpool, hpool, wpool, opool, pps, ppr,
          efT_d, basis_d, f_d, uw_sb, w1_sb, b1_sb, w2_sb, b2_sb,
          ones_sb, out_d):
    mdt = mybir.dt
    if True:
        for b in range(NBLK):
            # block loads
            efT_sb = inpool.tile([EDGE_DIM, EBLK], dt.float32r, tag="efT")
            nc.sync.dma_start(efT_sb[:], efT_d[:, b * EBLK:(b + 1) * EBLK])
            basis_sb = inpool.tile([128, BLK * 27], dt.float32, tag="basis")
            nc.sync.dma_start(basis_sb[:], basis_d[:, b * BLK * 27:(b + 1) * BLK * 27])
            fe_sb = inpool.tile([128, BLK, FPAD], dt.float32, tag="fe")
            for g in range(2):
                i0 = b * (EBLK // 16) + g * 64
                nc.gpsimd.dma_gather(
                    fe_sb[:, g * (BLK // 2):(g + 1) * (BLK // 2), :], f_d[:],
                    uw_sb[:, i0:i0 + 64],
                    num_idxs=EBLK // 2, num_idxs_reg=EBLK // 2, elem_size=FPAD,
                )

            # h.T = relu(W1.T @ efT + b1): [128h, EBLK]
            hT_sb = hpool.tile([HIDDEN, EBLK], dt.float32r, tag="hT")
            for q in range(EBLK // 512):
                hT_ps = pps.tile([HIDDEN, 512], dt.float32, tag="hTps")
                nc.tensor.matmul(hT_ps[:], w1_sb[:],
                                 efT_sb[:, q * 512:(q + 1) * 512],
                                 start=True, stop=True)
                nc.scalar.activation(hT_sb[:, q * 512:(q + 1) * 512], hT_ps[:],
                                     mybir.ActivationFunctionType.Relu,
                                     bias=b1_sb[:], scale=1.0)

            out_sb = opool.tile([128, BLK, 48], dt.float32, tag="outsb")

            for t in range(BLK):
                # rw = hT_chunk.T @ W2 + b2 : [128e, 768] in PSUM
                rw_ps = ppr.tile([128, RW], dt.float32, tag="rwps")
                hT_c = hT_sb[:, t * 128:(t + 1) * 128]
                nc.tensor.matmul(rw_ps[:, 0:512], hT_c, w2_sb[:, 0:512],
                                 start=True, stop=False)
                nc.tensor.matmul(rw_ps[:, 0:512], ones_sb[:], b2_sb[:, 0:512],
                                 start=False, stop=True)
                nc.tensor.matmul(rw_ps[:, 512:RW], hT_c, w2_sb[:, 512:RW],
                                 start=True, stop=False)
                nc.tensor.matmul(rw_ps[:, 512:RW], ones_sb[:], b2_sb[:, 512:RW],
                                 start=False, stop=True)

                # tmp[e, m, k] = sum_d fe[e, m, d] * basis[e, d, k]
                fe_t = fe_sb[:, t, 0:48].rearrange("p (m d) -> p m d", m=M1, d=D1)
                ba_t = basis_sb[:, t * 27:(t + 1) * 27].rearrange(
                    "p (d k) -> p d k", d=D1, k=NREPS * D2)
                tmp_sb = wpool.tile([128, M1, NREPS * D2], dt.float32, tag="tmp")
                tp_sb = wpool.tile([128, M1, NREPS * D2], dt.float32, tag="tprod")
                for d in range(D1):
                    fe_b = fe_t[:, :, d].unsqueeze(2).broadcast_to([128, M1, NREPS * D2])
                    ba_b = ba_t[:, d, :].unsqueeze(1).broadcast_to([128, M1, NREPS * D2])
                    dst = tmp_sb if d == 0 else tp_sb
                    nc.vector.tensor_tensor(dst[:], fe_b, ba_b, mybir.AluOpType.mult)
                    if d > 0:
                        nc.vector.tensor_tensor(tmp_sb[:], tmp_sb[:], tp_sb[:],
                                                mybir.AluOpType.add)

                # out[e, i, dd] = sum_{m,r} rw[e, i, m, r] * tmp[e, m, r, dd]
                rw4 = rw_ps[:].rearrange("p (i m r) -> p i m r", i=M2, m=M1, r=NREPS)
                tmp4 = tmp_sb[:].rearrange("p m (r dd) -> p m r dd", r=NREPS, dd=D2)
                out_t = out_sb[:, t, :].rearrange("p (i dd) -> p i dd", i=M2, dd=D2)
                for dd in range(D2):
                    t_b = tmp4[:, :, :, dd].unsqueeze(1).broadcast_to([128, M2, M1, NREPS])
                    prod = wpool.tile([128, M2, M1, NREPS], dt.float32, tag="prod")
                    nc.vector.tensor_tensor(prod[:], rw4, t_b, mybir.AluOpType.mult)
                    nc.vector.tensor_reduce(out_t[:, :, dd], prod[:],
                                            axis=mybir.AxisListType.XY,
                                            op=mybir.AluOpType.add)

            nc.sync.dma_start(out_d[:, b * BLK * 48:(b + 1) * BLK * 48],
                              out_sb[:].rearrange("p b k -> p (b k)"))


def _get_nc(reps=1):
    key = ("nc", reps)
    if key not in _CACHE:
        _CACHE[key] = _build(reps)
    return _CACHE[key]


def _prep_core(U_c, basis_c, ef_c, f, W1, b1, W2, b2):
    """Build one core's input map (host-side layout/swizzle)."""
    npad = ESP - U_c.shape[0]
    U_p = np.concatenate([np.asarray(U_c, np.int64), np.zeros(npad, np.int64)])
    basis_p = np.concatenate(
        [np.asarray(basis_c, np.float32).reshape(-1, 27),
         np.zeros((npad, 27), np.float32)], axis=0)
    ef_p = np.concatenate(
        [np.asarray(ef_c, np.float32),
         np.zeros((npad, EDGE_DIM), np.float32)], axis=0)

    efT = np.ascontiguousarray(ef_p.T)                                   # [32, ESP]
    basisp = np.ascontiguousarray(
        basis_p.reshape(NTILES, 128, 27).transpose(1, 0, 2).reshape(128, NTILES * 27))
    uw16 = U_p.astype(np.int16).reshape(NBLK, 128, 16).transpose(0, 2, 1)  # [NBLK,16,128]
    uw = np.ascontiguousarray(
        np.tile(uw16.transpose(1, 0, 2).reshape(16, ESP // 16), (8, 1)))  # [128, ESP//16]
    fpad = np.zeros((N, FPAD), np.float32)
    fpad[:, :M1 * D1] = np.asarray(f, np.float32).reshape(N, M1 * D1)
    return {
        "efT": efT,
        "basisp": basisp,
        "uw": uw,
        "fpad": fpad,
        "w1": np.asarray(W1, np.float32),
        "b1": np.asarray(b1, np.float32).reshape(HIDDEN, 1),
        "w2": np.asarray(W2, np.float32),
        "b2": np.asarray(b2, np.float32).reshape(1, RW),
        "ones1": np.ones((1, 128), np.float32),
    }


def kernel(U, basis, edge_feats, f, W1, b1, W2, b2):
    U = np.asarray(U)
    basis = np.asarray(basis, np.float32)
    edge_feats = np.asarray(edge_feats, np.float32)
    nc = _get_nc()
    in_maps = []
    for c in range(NCORES):
        sl = slice(c * ES, (c + 1) * ES)
        in_maps.append(_prep_core(U[sl], basis[sl], edge_feats[sl],
                                  f, W1, b1, W2, b2))
    res = run_bass_kernel_spmd(nc, in_maps, core_ids=list(range(NCORES)))
    outs = []
    for c in range(NCORES):
        op = res.results[c]["outp"]                                   # [128, NTILES*48]
        o = op.reshape(128, NTILES, 48).transpose(1, 0, 2).reshape(ESP, 48)
        outs.append(o[:ES])
    return np.concatenate(outs, axis=0).reshape(E, M2, D2).astype(np.float32)


if __name__ == "__main__":
    # quick self-run with random data
    rng = np.random.default_rng(0)
    inputs = {
        "U": rng.integers(0, N, size=E),
        "basis": rng.standard_normal((E, D1, NREPS * D2), dtype=np.float32),
        "edge_feats": rng.standard_normal((E, EDGE_DIM), dtype=np.float32),
        "f": rng.standard_normal((N, M1, D1), dtype=np.float32),
        "W1": (rng.standard_normal((EDGE_DIM, HIDDEN), dtype=np.float32) / np.sqrt(EDGE_DIM)),
        "b1": rng.standard_normal(HIDDEN, dtype=np.float32) * 0.02,
        "W2": (rng.standard_normal((HIDDEN, RW), dtype=np.float32) / np.sqrt(HIDDEN)),
        "b2": rng.standard_normal(RW, dtype=np.float32) * 0.02,
    }
    out = kernel(**inputs)
    print(out.shape, out.dtype)


# revision 7
# speedup vs baseline: 1.1855x; 1.1855x over previous
"""Trainium2 Bass kernel for nn_EquivariantConvolution (gnn_message_passing).

Math (per edge e):
    h  = relu(edge_feats @ W1 + b1)            [E,128]
    rw = (h @ W2 + b2) -> [E, 16, 48]
    fe = f[U]                                  [E,16,3]
    tmp[e,m,k] = sum_d fe[e,m,d] * basis[e,d,k]        (k = r*3+dd, 9)
    out[e,i,dd] = sum_{m,r} rw[e,i,m*3+r] * tmp[e,m,r*3+dd]

Sharding: edges split across 8 cores (40000 each, padded to 40960);
f + MLP weights replicated. Device layout: edge j of a 128-edge tile on
SBUF partition j%128; MLP on the tensor engine (float32r), f[U] via
dma_gather, per-edge contractions on the vector engine with broadcast APs.
"""
import sys

sys.path.insert(0, "/opt/trn_rl_repo")

import numpy as np
import concourse.bass as bass
import concourse.bacc as bacc
import concourse.mybir as mybir
import concourse.tile as tile
from concourse.bass_utils import run_bass_kernel_spmd
from contextlib import ExitStack

# problem constants (hardcoded per harness contract)
E = 320000
N = 10000
M1 = 16
M2 = 16
D1 = 3
D2 = 3
NREPS = 3
EDGE_DIM = 32
HIDDEN = 128
RW = NREPS * M1 * M2  # 768

NCORES = 8
ES = E // NCORES          # 40000 edges per core
ESP = 40960               # padded to 320 tiles of 128
NTILES = ESP // 128       # 320
BLK = 16                  # tiles per block
NBLK = NTILES // BLK      # 20
EBLK = BLK * 128          # 2048 edges per block
FPAD = 64                 # f rows padded to 64 floats (256B) for dma_gather

_CACHE = {}


def _build(reps=1):
    dt = mybir.dt
    nc = bacc.Bacc("TRN2", target_bir_lowering=False, debug=False,
                   num_devices=NCORES)

    # DRAM tensors (per-core shards fed via in_maps)
    efT_d = nc.dram_tensor("efT", [EDGE_DIM, ESP], dt.float32r, kind="ExternalInput").ap()
    basis_d = nc.dram_tensor("basisp", [128, NTILES * 27], dt.float32, kind="ExternalInput").ap()
    uw_d = nc.dram_tensor("uw", [128, ESP // 16], dt.int16, kind="ExternalInput").ap()
    f_d = nc.dram_tensor("fpad", [N, FPAD], dt.float32, kind="ExternalInput").ap()
    w1_d = nc.dram_tensor("w1", [EDGE_DIM, HIDDEN], dt.float32r, kind="ExternalInput").ap()
    b1_d = nc.dram_tensor("b1", [HIDDEN, 1], dt.float32, kind="ExternalInput").ap()
    w2_d = nc.dram_tensor("w2", [HIDDEN, RW], dt.float32r, kind="ExternalInput").ap()
    b2_d = nc.dram_tensor("b2", [1, RW], dt.float32r, kind="ExternalInput").ap()
    ones_d = nc.dram_tensor("ones1", [1, 128], dt.float32r, kind="ExternalInput").ap()
    out_d = nc.dram_tensor("outp", [128, NTILES * 48], dt.float32, kind="ExternalOutput").ap()

    with tile.TileContext(nc) as tc, ExitStack() as ctx:
        cpool = ctx.enter_context(tc.tile_pool(name="const", bufs=1))
        inpool = ctx.enter_context(tc.tile_pool(name="in", bufs=3))
        hpool = ctx.enter_context(tc.tile_pool(name="h", bufs=2))
        wpool = ctx.enter_context(tc.tile_pool(name="work", bufs=4))
        opool = ctx.enter_context(tc.tile_pool(name="out", bufs=2))
        pps = ctx.enter_context(tc.tile_pool(name="psA", bufs=2, space="PSUM"))
        ppr = ctx.enter_context(tc.tile_pool(name="psB", bufs=3, space="PSUM"))

        # constants
        w1_sb = cpool.tile([EDGE_DIM, HIDDEN], dt.float32r)
        nc.sync.dma_start(w1_sb[:], w1_d[:])
        b1_sb = cpool.tile([HIDDEN, 1], dt.float32)
        nc.sync.dma_start(b1_sb[:], b1_d[:])
        w2_sb = cpool.tile([HIDDEN, RW], dt.float32r)
        nc.sync.dma_start(w2_sb[:], w2_d[:])
        b2_sb = cpool.tile([1, RW], dt.float32r)
        nc.sync.dma_start(b2_sb[:], b2_d[:])
        ones_sb = cpool.tile([1, 128], dt.float32r)
        nc.sync.dma_start(ones_sb[:], ones_d[:])
        uw_sb = cpool.tile([128, ESP // 16], dt.int16)
        nc.sync.dma_start(uw_sb[:], uw_d[:])

        def body():
            _body(nc, tc, dt, cpool, inpool, hpool, wpool, opool, pps, ppr,
                  efT_d, basis_d, f_d, uw_sb, w1_sb, b1_sb, w2_sb, b2_sb,
                  ones_sb, out_d)

        if reps == 1:
            body()
        else:
            with tc.For_i(0, reps, 1):
                body()

    nc.compile()
    return nc


def _body(nc, tc, dt, cpool, inpool, hpool, wpool, opool, pps, ppr,
          efT_d, basis_d, f_d, uw_sb, w1_sb, b1_sb, w2_sb, b2_sb,
          ones_sb, out_d):
    mdt = mybir.dt
    if True:
        for b in range(NBLK):
            # block loads
            efT_sb = inpool.tile([EDGE_DIM, EBLK], dt.float32r, tag="efT")
            nc.sync.dma_start(efT_sb[:], efT_d[:, b * EBLK:(b + 1) * EBLK])
            basis_sb = inpool.tile([128, BLK * 27], dt.float32, tag="basis")
            nc.sync.dma_start(basis_sb[:], basis_d[:, b * BLK * 27:(b + 1) * BLK * 27])
            fe_sb = inpool.tile([128, BLK, FPAD], dt.float32, tag="fe")
            for g in range(2):
                i0 = b * (EBLK // 16) + g * 64
                nc.gpsimd.dma_gather(
                    fe_sb[:, g * (BLK // 2):(g + 1) * (BLK // 2), :], f_d[:],
                    uw_sb[:, i0:i0 + 64],
                    num_idxs=EBLK // 2, num_idxs_reg=EBLK // 2, elem_size=FPAD,
                )

            # h.T = relu(W1.T @ efT + b1): [128h, EBLK]
            hT_sb = hpool.tile([HIDDEN, EBLK], dt.float32r, tag="hT")
            for q in range(EBLK // 512):
                hT_ps = pps.tile([HIDDEN, 512], dt.float32, tag="hTps")
                nc.tensor.matmul(hT_ps[:], w1_sb[:],
                                 efT_sb[:, q * 512:(q + 1) * 512],
                                 start=True, stop=True)
                nc.scalar.activation(hT_sb[:, q * 512:(q + 1) * 512], hT_ps[:],
                                     mybir.ActivationFunctionType.Relu,
                                     bias=b1_sb[:], scale=1.0)

            out_sb = opool.tile([128, BLK, 48], dt.float32, tag="outsb")

            for t in range(BLK):
                # rw = hT_chunk.T @ W2 + b2 : [128e, 768] in PSUM
                rw_ps = ppr.tile([128, RW], dt.float32, tag="rwps")
                hT_c = hT_sb[:, t * 128:(t + 1) * 128]
                nc.tensor.matmul(rw_ps[:, 0:512], hT_c, w2_sb[:, 0:512],
                                 start=True, stop=False)
                nc.tensor.matmul(rw_ps[:, 0:512], ones_sb[:], b2_sb[:, 0:512],
                                 start=False, stop=True)
                nc.tensor.matmul(rw_ps[:, 512:RW], hT_c, w2_sb[:, 512:RW],
                                 start=True, stop=False)
                nc.tensor.matmul(rw_ps[:, 512:RW], ones_sb[:], b2_sb[:, 512:RW],
                                 start=False, stop=True)

                # tmp[e, m, k] = sum_d fe[e, m, d] * basis[e, d, k]
                fe_t = fe_sb[:, t, 0:48].rearrange("p (m d) -> p m d", m=M1, d=D1)
                ba_t = basis_sb[:, t * 27:(t + 1) * 27].rearrange(
                    "p (d k) -> p d k", d=D1, k=NREPS * D2)
                tmp_sb = wpool.tile([128, M1, NREPS * D2], dt.float32, tag="tmp")
                tp_sb = wpool.tile([128, M1, NREPS * D2], dt.float32, tag="tprod")
                for d in range(D1):
                    fe_b = fe_t[:, :, d].unsqueeze(2).broadcast_to([128, M1, NREPS * D2])
                    ba_b = ba_t[:, d, :].unsqueeze(1).broadcast_to([128, M1, NREPS * D2])
                    dst = tmp_sb if d == 0 else tp_sb
                    nc.vector.tensor_tensor(dst[:], fe_b, ba_b, mybir.AluOpType.mult)
                    if d > 0:
                        nc.vector.tensor_tensor(tmp_sb[:], tmp_sb[:], tp_sb[:],
                                                mybir.AluOpType.add)

                # rw also to SBUF (via ACT) so GPSIMD can take one dd slice
                rw_sb = wpool.tile([128, RW], dt.float32, tag="rwsb")
                nc.scalar.activation(rw_sb[:], rw_ps[:],
                                     mybir.ActivationFunctionType.Identity,
                                     bias=0.0, scale=1.0)

                # out[e, i, dd] = sum_{m,r} rw[e, i, m, r] * tmp[e, m, r, dd]
                rw4 = rw_ps[:].rearrange("p (i m r) -> p i m r", i=M2, m=M1, r=NREPS)
                rw4s = rw_sb[:].rearrange("p (i m r) -> p i m r", i=M2, m=M1, r=NREPS)
                tmp4 = tmp_sb[:].rearrange("p m (r dd) -> p m r dd", r=NREPS, dd=D2)
                out_t = out_sb[:, t, :].rearrange("p (i dd) -> p i dd", i=M2, dd=D2)
                for dd in range(D2):
                    t_b = tmp4[:, :, :, dd].unsqueeze(1).broadcast_to([128, M2, M1, NREPS])
                    if dd < 2:
                        prod = wpool.tile([128, M2, M1, NREPS], dt.float32, tag="prod")
                        nc.vector.tensor_tensor(prod[:], rw4, t_b, mybir.AluOpType.mult)
                        nc.vector.tensor_reduce(out_t[:, :, dd], prod[:],
                                                axis=mybir.AxisListType.XY,
                                                op=mybir.AluOpType.add)
                    else:
                        # GPSIMD path: mult then pairwise tree-add over (m,r)=48
                        prodg = wpool.tile([128, M2, M1 * NREPS], dt.float32, tag="prodg")
                        nc.gpsimd.tensor_tensor(
                            prodg[:].rearrange("p i (m r) -> p i m r", m=M1, r=NREPS),
                            rw4s, t_b, mybir.AluOpType.mult)
                        nc.gpsimd.tensor_tensor(prodg[:, :, 0:24], prodg[:, :, 0:24],
                                                prodg[:, :, 24:48], mybir.AluOpType.add)
                        nc.gpsimd.tensor_tensor(prodg[:, :, 0:12], prodg[:, :, 0:12],
                                                prodg[:, :, 12:24], mybir.AluOpType.add)
                        nc.gpsimd.tensor_tensor(prodg[:, :, 0:6], prodg[:, :, 0:6],
                                                prodg[:, :, 6:12], mybir.AluOpType.add)
                        nc.gpsimd.tensor_tensor(prodg[:, :, 0:3], prodg[:, :, 0:3],
                                                prodg[:, :, 3:6], mybir.AluOpType.add)
                        nc.gpsimd.tensor_tensor(prodg[:, :, 0:1], prodg[:, :, 0:1],
                                                prodg[:, :, 1:2], mybir.AluOpType.add)
                        nc.gpsimd.tensor_tensor(out_t[:, :, dd].unsqueeze(2),
                                                prodg[:, :, 0:1], prodg[:, :, 2:3],
                                                mybir.AluOpType.add)

            nc.sync.dma_start(out_d[:, b * BLK * 48:(b + 1) * BLK * 48],
                              out_sb[:].rearrange("p b k -> p (b k)"))


def _get_nc(reps=1):
    key = ("nc", reps)
    if key not in _CACHE:
        _CACHE[key] = _build(reps)
    return _CACHE[key]


def _prep_core(U_c, basis_c, ef_c, f, W1, b1, W2, b2):
    """Build one core's input map (host-side layout/swizzle)."""
    npad = ESP - U_c.shape[0]
    U_p = np.concatenate([np.asarray(U_c, np.int64), np.zeros(npad, np.int64)])
    basis_p = np.concatenate(
        [np.asarray(basis_c, np.float32).reshape(-1, 27),
         np.zeros((npad, 27), np.float32)], axis=0)
    ef_p = np.concatenate(
        [np.asarray(ef_c, np.float32),
         np.zeros((npad, EDGE_DIM), np.float32)], axis=0)

    efT = np.ascontiguousarray(ef_p.T)                                   # [32, ESP]
    basisp = np.ascontiguousarray(
        basis_p.reshape(NTILES, 128, 27).transpose(1, 0, 2).reshape(128, NTILES * 27))
    uw16 = U_p.astype(np.int16).reshape(NBLK, 128, 16).transpose(0, 2, 1)  # [NBLK,16,128]
    uw = np.ascontiguousarray(
        np.tile(uw16.transpose(1, 0, 2).reshape(16, ESP // 16), (8, 1)))  # [128, ESP//16]
    fpad = np.zeros((N, FPAD), np.float32)
    fpad[:, :M1 * D1] = np.asarray(f, np.float32).reshape(N, M1 * D1)
    return {
        "efT": efT,
        "basisp": basisp,
        "uw": uw,
        "fpad": fpad,
        "w1": np.asarray(W1, np.float32),
        "b1": np.asarray(b1, np.float32).reshape(HIDDEN, 1),
        "w2": np.asarray(W2, np.float32),
        "b2": np.asarray(b2, np.float32).reshape(1, RW),
        "ones1": np.ones((1, 128), np.float32),
    }


def kernel(U, basis, edge_feats, f, W1, b1, W2, b2):
    U = np.asarray(U)
    basis = np.asarray(basis, np.float32)
    edge_feats = np.asarray(edge_feats, np.float32)
    nc = _get_nc()
    in_maps = []
    for c in range(NCORES):
        sl = slice(c * ES, (c + 1) * ES)
        in_maps.append(_prep_core(U[sl], basis[sl], edge_feats[sl],
                                  f, W1, b1, W2, b2))
    res = run_bass_kernel_spmd(nc, in_maps, core_ids=list(range(NCORES)))
    outs = []
    for c in range(NCORES):
        op = res.results[c]["outp"]                                   # [128, NTILES*48]
        o = op.reshape(128, NTILES, 48).transpose(1, 0, 2).reshape(ESP, 48)
        outs.append(o[:ES])
    return np.concatenate(outs, axis=0).reshape(E, M2, D2).astype(np.float32)


if __name__ == "__main__":
    # quick self-run with random data
    rng = np.random.default_rng(0)
    inputs = {
        "U": rng.integers(0, N, size=E),
        "basis": rng.standard_normal((E, D1, NREPS * D2), dtype=np.float32),
        "edge_feats": rng.standard_normal((E, EDGE_DIM), dtype=np.float32),
        "f": rng.standard_normal((N, M1, D1), dtype=np.float32),
        "W1": (rng.standard_normal((EDGE_DIM, HIDDEN), dtype=np.float32) / np.sqrt(EDGE_DIM)),
        "b1": rng.standard_normal(HIDDEN, dtype=np.float32) * 0.02,
        "W2": (rng.standard_normal((HIDDEN, RW), dtype=np.float32) / np.sqrt(HIDDEN)),
        "b2": rng.standard_normal(RW, dtype=np.float32) * 0.02,
    }
    out = kernel(**inputs)
    print(out.shape, out.dtype)


# revision 24
# speedup vs baseline: 1.5850x; 1.3370x over previous
"""Trainium2 Bass kernel for nn_EquivariantConvolution (gnn_message_passing).

Math (per edge e):
    h  = relu(edge_feats @ W1 + b1)            [E,128]
    rw = (h @ W2 + b2) -> [E, 16, 48]
    fe = f[U]                                  [E,16,3]
    tmp[e,m,k] = sum_d fe[e,m,d] * basis[e,d,k]        (k = r*3+dd, 9)
    out[e,i,dd] = sum_{m,r} rw[e,i,m*3+r] * tmp[e,m,r*3+dd]

Sharding: edges split across 8 cores (40000 each, padded to 40960);
f + MLP weights replicated. Device layout: edge j of a 128-edge tile on
SBUF partition j%128; MLP on the tensor engine (float32r), f[U] via
dma_gather, per-edge contractions on the vector engine with broadcast APs.
"""
import sys

sys.path.insert(0, "/opt/trn_rl_repo")

import os
import numpy as np
import concourse.bass as bass
import concourse.bacc as bacc
import concourse.mybir as mybir
import concourse.tile as tile
from concourse.bass_utils import run_bass_kernel_spmd
from contextlib import ExitStack

# problem constants (hardcoded per harness contract)
E = 320000
N = 10000
M1 = 16
M2 = 16
D1 = 3
D2 = 3
NREPS = 3
EDGE_DIM = 32
HIDDEN = 128
RW = NREPS * M1 * M2  # 768

NCORES = 8
ES = E // NCORES          # 40000 edges per core
ESP = 40960               # padded to 320 tiles of 128
NTILES = ESP // 128       # 320
BLK = int(os.environ.get('KBLK', '16'))        # tiles per block
NBLK = NTILES // BLK      # 20
EBLK = BLK * 128          # 2048 edges per block
FPAD = 64                 # f rows padded to 64 floats (256B) for dma_gather

_CACHE = {}


ABL = set(os.environ.get("KABL", "").split(","))  # ablation flags for benching


def _build(reps=1):
    dt = mybir.dt
    nc = bacc.Bacc("TRN2", target_bir_lowering=False, debug=False,
                   num_devices=NCORES)

    # DRAM tensors (per-core shards fed via in_maps)
    efT_d = nc.dram_tensor("efT", [EDGE_DIM, ESP], dt.float32r, kind="ExternalInput").ap()
    basis_d = nc.dram_tensor("basisp", [128, NTILES * 27], dt.float32, kind="ExternalInput").ap()
    uw_d = nc.dram_tensor("uw", [128, ESP // 16], dt.int16, kind="ExternalInput").ap()
    f_d = nc.dram_tensor("fpad", [N, FPAD], dt.float32, kind="ExternalInput").ap()
    w1_d = nc.dram_tensor("w1", [EDGE_DIM, HIDDEN], dt.float32r, kind="ExternalInput").ap()
    b1_d = nc.dram_tensor("b1", [HIDDEN, 1], dt.float32, kind="ExternalInput").ap()
    w2_d = nc.dram_tensor("w2", [HIDDEN, RW], dt.float32r, kind="ExternalInput").ap()
    b2_d = nc.dram_tensor("b2", [1, RW], dt.float32r, kind="ExternalInput").ap()
    ones_d = nc.dram_tensor("ones1", [1, 128], dt.float32r, kind="ExternalInput").ap()
    out_d = nc.dram_tensor("outp", [128, NTILES * 48], dt.float32, kind="ExternalOutput").ap()

    with tile.TileContext(nc) as tc, ExitStack() as ctx:
        cpool = ctx.enter_context(tc.tile_pool(name="const", bufs=1))
        inpool = ctx.enter_context(tc.tile_pool(name="in", bufs=3))
        hpool = ctx.enter_context(tc.tile_pool(name="h", bufs=2))
        wpool = ctx.enter_context(tc.tile_pool(name="work", bufs=4))
        opool = ctx.enter_context(tc.tile_pool(name="out", bufs=2))
        pps = ctx.enter_context(tc.tile_pool(name="psA", bufs=2, space="PSUM"))
        ppr = ctx.enter_context(tc.tile_pool(name="psB", bufs=3, space="PSUM"))

        # constants
        w1_sb = cpool.tile([EDGE_DIM, HIDDEN], dt.float32r)
        nc.sync.dma_start(w1_sb[:], w1_d[:])
        b1_sb = cpool.tile([HIDDEN, 1], dt.float32)
        nc.sync.dma_start(b1_sb[:], b1_d[:])
        w2_sb = cpool.tile([HIDDEN, RW], dt.float32r)
        nc.sync.dma_start(w2_sb[:], w2_d[:])
        b2_sb = cpool.tile([1, RW], dt.float32r)
        nc.sync.dma_start(b2_sb[:], b2_d[:])
        ones_sb = cpool.tile([1, 128], dt.float32r)
        nc.sync.dma_start(ones_sb[:], ones_d[:])
        uw_sb = cpool.tile([128, ESP // 16], dt.int16)
        nc.sync.dma_start(uw_sb[:], uw_d[:])

        def body():
            _body(nc, tc, dt, cpool, inpool, hpool, wpool, opool, pps, ppr,
                  efT_d, basis_d, f_d, uw_sb, w1_sb, b1_sb, w2_sb, b2_sb,
                  ones_sb, out_d)

        if reps == 1:
            body()
        else:
            with tc.For_i(0, reps, 1):
                body()

    nc.compile()
    return nc


def _body(nc, tc, dt, cpool, inpool, hpool, wpool, opool, pps, ppr,
          efT_d, basis_d, f_d, uw_sb, w1_sb, b1_sb, w2_sb, b2_sb,
          ones_sb, out_d):
    mdt = mybir.dt
    if True:
        for b in range(NBLK):
            # block loads
            efT_sb = inpool.tile([EDGE_DIM, EBLK], dt.float32r, tag="efT")
            nc.sync.dma_start(efT_sb[:], efT_d[:, b * EBLK:(b + 1) * EBLK])
            basis_sb = inpool.tile([128, BLK * 27], dt.float32, tag="basis")
            nc.sync.dma_start(basis_sb[:], basis_d[:, b * BLK * 27:(b + 1) * BLK * 27])
            fe_sb = inpool.tile([128, BLK, FPAD], dt.float32, tag="fe")
            for g in range(EBLK // 1024):
                i0 = b * (EBLK // 16) + g * 64
                nc.gpsimd.dma_gather(
                    fe_sb[:, g * 8:(g + 1) * 8, :], f_d[:],
                    uw_sb[:, i0:i0 + 64],
                    num_idxs=1024, num_idxs_reg=1024, elem_size=FPAD,
                )

            # h.T = relu(W1.T @ efT + b1): [128h, EBLK]
            hT_sb = hpool.tile([HIDDEN, EBLK], dt.float32r, tag="hT")
            for q in range(EBLK // 512 if "nomlp" not in ABL else 0):
                hT_ps = pps.tile([HIDDEN, 512], dt.float32, tag="hTps")
                nc.tensor.matmul(hT_ps[:], w1_sb[:],
                                 efT_sb[:, q * 512:(q + 1) * 512],
                                 start=True, stop=True)
                nc.scalar.activation(hT_sb[:, q * 512:(q + 1) * 512], hT_ps[:],
                                     mybir.ActivationFunctionType.Relu,
                                     bias=b1_sb[:], scale=1.0)

            out_sb = opool.tile([128, BLK, 48], dt.float32, tag="outsb")

            for t in range(BLK):
                # rw = hT_chunk.T @ W2 + b2 : [128e, 768] in PSUM
                rw_ps = ppr.tile([128, RW], dt.float32, tag="rwps")
                hT_c = hT_sb[:, t * 128:(t + 1) * 128]
                if "nomlp" not in ABL:
                    nc.tensor.matmul(rw_ps[:, 0:512], hT_c, w2_sb[:, 0:512],
                                     start=True, stop=False)
                    nc.tensor.matmul(rw_ps[:, 0:512], ones_sb[:], b2_sb[:, 0:512],
                                     start=False, stop=True)
                    nc.tensor.matmul(rw_ps[:, 512:RW], hT_c, w2_sb[:, 512:RW],
                                     start=True, stop=False)
                    nc.tensor.matmul(rw_ps[:, 512:RW], ones_sb[:], b2_sb[:, 512:RW],
                                     start=False, stop=True)

                # tmp[e, m, k] = sum_d fe[e, m, d] * basis[e, d, k]
                fe_t = fe_sb[:, t, 0:48].rearrange("p (m d) -> p m d", m=M1, d=D1)
                ba_t = basis_sb[:, t * 27:(t + 1) * 27].rearrange(
                    "p (d k) -> p d k", d=D1, k=NREPS * D2)
                tmp_sb = wpool.tile([128, M1, NREPS * D2], dt.float32, tag="tmp")
                if "notmp" not in ABL:
                    # one fused mult over (m,k,d) then reduce innermost d;
                    # alternate tiles go to GPSIMD (otherwise idle) as
                    # mult + two strided adds (no X-reduce on POOL).
                    K9 = NREPS * D2
                    tpd = wpool.tile([128, M1, K9, D1], dt.float32, tag="tprodd")
                    fe_b = fe_t.unsqueeze(2).broadcast_to([128, M1, K9, D1])  # [p,m,k*,d]
                    ba_kd = basis_sb[:, t * 27:(t + 1) * 27].rearrange(
                        "p (d k) -> p k d", d=D1, k=K9)
                    ba_b = ba_kd.unsqueeze(1).broadcast_to([128, M1, K9, D1])  # [p,m*,k,d]
                    nc.vector.tensor_tensor(tpd[:], fe_b, ba_b, mybir.AluOpType.mult)
                    nc.vector.tensor_reduce(tmp_sb[:], tpd[:],
                                            axis=mybir.AxisListType.X,
                                            op=mybir.AluOpType.add)



                # out[e, i, dd] = sum_{m,r} rw[e, i, m, r] * tmp[e, m, r, dd]
                # fused: one mult into (i, dd, m, r) layout, one XY-reduce
                if "nostep5" not in ABL:
                    rw_b = rw_ps[:].rearrange(
                        "p (i m r) -> p i m r", i=M2, m=M1, r=NREPS
                    ).unsqueeze(2).broadcast_to([128, M2, D2, M1, NREPS])
                    tmp_b = tmp_sb[:].rearrange(
                        "p m (r dd) -> p dd m r", r=NREPS, dd=D2
                    ).unsqueeze(1).broadcast_to([128, M2, D2, M1, NREPS])
                    prod = wpool.tile([128, M2 * D2, M1, NREPS], dt.float32, tag="prod")
                    nc.vector.tensor_tensor(
                        prod[:].rearrange("p (i dd) m r -> p i dd m r", i=M2, dd=D2),
                        rw_b, tmp_b, mybir.AluOpType.mult)
                    nc.vector.tensor_reduce(out_sb[:, t, :], prod[:],
                                            axis=mybir.AxisListType.XY,
                                            op=mybir.AluOpType.add)

            if "nostep5" not in ABL:
                nc.sync.dma_start(out_d[:, b * BLK * 48:(b + 1) * BLK * 48],
                                  out_sb[:].rearrange("p b k -> p (b k)"))


def _get_nc(reps=1):
    key = ("nc", reps)
    if key not in _CACHE:
        _CACHE[key] = _build(reps)
    return _CACHE[key]


def _prep_core(U_c, basis_c, ef_c, f, W1, b1, W2, b2):
    """Build one core's input map (host-side layout/swizzle)."""
    npad = ESP - U_c.shape[0]
    U_p = np.concatenate([np.asarray(U_c, np.int64), np.zeros(npad, np.int64)])
    basis_p = np.concatenate(
        [np.asarray(basis_c, np.float32).reshape(-1, 27),
         np.zeros((npad, 27), np.float32)], axis=0)
    ef_p = np.concatenate(
        [np.asarray(ef_c, np.float32),
         np.zeros((npad, EDGE_DIM), np.float32)], axis=0)

    efT = np.ascontiguousarray(ef_p.T)                                   # [32, ESP]
    basisp = np.ascontiguousarray(
        basis_p.reshape(NTILES, 128, 27).transpose(1, 0, 2).reshape(128, NTILES * 27))
    # wrapped per 1024-index gather group: uw[p, c] = U[(c//64)*1024 + (c%64)*16 + p]
    uw16 = U_p.astype(np.int16).reshape(ESP // 1024, 64, 16).transpose(2, 0, 1)
    uw = np.ascontiguousarray(
        np.tile(uw16.reshape(16, ESP // 16), (8, 1)))  # [128, ESP//16], 8x for Q7 cores
    fpad = np.zeros((N, FPAD), np.float32)
    fpad[:, :M1 * D1] = np.asarray(f, np.float32).reshape(N, M1 * D1)
    return {
        "efT": efT,
        "basisp": basisp,
        "uw": uw,
        "fpad": fpad,
        "w1": np.asarray(W1, np.float32),
        "b1": np.asarray(b1, np.float32).reshape(HIDDEN, 1),
        "w2": np.asarray(W2, np.float32),
        "b2": np.asarray(b2, np.float32).reshape(1, RW),
        "ones1": np.ones((1, 128), np.float32),
    }


def kernel(U, basis, edge_feats, f, W1, b1, W2, b2):
    U = np.asarray(U)
    basis = np.asarray(basis, np.float32)
    edge_feats = np.asarray(edge_feats, np.float32)
    nc = _get_nc()
    in_maps = []
    for c in range(NCORES):
        sl = slice(c * ES, (c + 1) * ES)
        in_maps.append(_prep_core(U[sl], basis[sl], edge_feats[sl],
                                  f, W1, b1, W2, b2))
    res = run_bass_kernel_spmd(nc, in_maps, core_ids=list(range(NCORES)))
    outs = []
    for c in range(NCORES):
        op = res.results[c]["outp"]                                   # [128, NTILES*48]
        o = op.reshape(128, NTILES, 48).transpose(1, 0, 2).reshape(ESP, 48)
        outs.append(o[:ES])
    return np.concatenate(outs, axis=0).reshape(E, M2, D2).astype(np.float32)


if __name__ == "__main__":
    # quick self-run with random data
    rng = np.random.default_rng(0)
    inputs = {
        "U": rng.integers(0, N, size=E),
        "basis": rng.standard_normal((E, D1, NREPS * D2), dtype=np.float32),
        "edge_feats": rng.standard_normal((E, EDGE_DIM), dtype=np.float32),
        "f": rng.standard_normal((N, M1, D1), dtype=np.float32),
        "W1": (rng.standard_normal((EDGE_DIM, HIDDEN), dtype=np.float32) / np.sqrt(EDGE_DIM)),
        "b1": rng.standard_normal(HIDDEN, dtype=np.float32) * 0.02,
        "W2": (rng.standard_normal((HIDDEN, RW), dtype=np.float32) / np.sqrt(HIDDEN)),
        "b2": rng.standard_normal(RW, dtype=np.float32) * 0.02,
    }
    out = kernel(**inputs)
    print(out.shape, out.dtype)


# revision 25
# speedup vs baseline: 1.6036x; 1.0117x over previous
"""Trainium2 Bass kernel for nn_EquivariantConvolution (gnn_message_passing).

Math (per edge e):
    h  = relu(edge_feats @ W1 + b1)            [E,128]
    rw = (h @ W2 + b2) -> [E, 16, 48]
    fe = f[U]                                  [E,16,3]
    tmp[e,m,k] = sum_d fe[e,m,d] * basis[e,d,k]        (k = r*3+dd, 9)
    out[e,i,dd] = sum_{m,r} rw[e,i,m*3+r] * tmp[e,m,r*3+dd]

Sharding: edges split across 8 cores (40000 each, padded to 40960);
f + MLP weights replicated. Device layout: edge j of a 128-edge tile on
SBUF partition j%128; MLP on the tensor engine (float32r), f[U] via
dma_gather, per-edge contractions on the vector engine with broadcast APs.
"""
import sys

sys.path.insert(0, "/opt/trn_rl_repo")

import os
import numpy as np
import concourse.bass as bass
import concourse.bacc as bacc
import concourse.mybir as mybir
import concourse.tile as tile
from concourse.bass_utils import run_bass_kernel_spmd
from contextlib import ExitStack

# problem constants (hardcoded per harness contract)
E = 320000
N = 10000
M1 = 16
M2 = 16
D1 = 3
D2 = 3
NREPS = 3
EDGE_DIM = 32
HIDDEN = 128
RW = NREPS * M1 * M2  # 768

NCORES = 8
ES = E // NCORES          # 40000 edges per core
ESP = 40960               # padded to 320 tiles of 128
NTILES = ESP // 128       # 320
BLK = int(os.environ.get('KBLK', '16'))        # tiles per block
NBLK = NTILES // BLK      # 20
EBLK = BLK * 128          # 2048 edges per block
FPAD = 64                 # f rows padded to 64 floats (256B) for dma_gather

_CACHE = {}


ABL = set(os.environ.get("KABL", "").split(","))  # ablation flags for benching


def _build(reps=1):
    dt = mybir.dt
    nc = bacc.Bacc("TRN2", target_bir_lowering=False, debug=False,
                   num_devices=NCORES)

    # DRAM tensors (per-core shards fed via in_maps)
    efT_d = nc.dram_tensor("efT", [EDGE_DIM, ESP], dt.float32r, kind="ExternalInput").ap()
    basis_d = nc.dram_tensor("basisp", [128, NTILES * 27], dt.float32, kind="ExternalInput").ap()
    uw_d = nc.dram_tensor("uw", [128, ESP // 16], dt.int16, kind="ExternalInput").ap()
    f_d = nc.dram_tensor("fpad", [N, FPAD], dt.float32, kind="ExternalInput").ap()
    w1_d = nc.dram_tensor("w1", [EDGE_DIM, HIDDEN], dt.float32r, kind="ExternalInput").ap()
    b1_d = nc.dram_tensor("b1", [HIDDEN, 1], dt.float32, kind="ExternalInput").ap()
    w2_d = nc.dram_tensor("w2", [HIDDEN, RW], dt.float32r, kind="ExternalInput").ap()
    b2_d = nc.dram_tensor("b2", [1, RW], dt.float32r, kind="ExternalInput").ap()
    ones_d = nc.dram_tensor("ones1", [1, 128], dt.float32r, kind="ExternalInput").ap()
    out_d = nc.dram_tensor("outp", [128, NTILES * 48], dt.float32, kind="ExternalOutput").ap()

    with tile.TileContext(nc) as tc, ExitStack() as ctx:
        BUFS = int(os.environ.get("KBUFS", "0"))
        cpool = ctx.enter_context(tc.tile_pool(name="const", bufs=1))
        inpool = ctx.enter_context(tc.tile_pool(name="in", bufs=4 if BUFS else 3))
        hpool = ctx.enter_context(tc.tile_pool(name="h", bufs=3 if BUFS else 2))
        wpool = ctx.enter_context(tc.tile_pool(name="work", bufs=6 if BUFS else 4))
        opool = ctx.enter_context(tc.tile_pool(name="out", bufs=3 if BUFS else 2))
        pps = ctx.enter_context(tc.tile_pool(name="psA", bufs=2, space="PSUM"))
        ppr = ctx.enter_context(tc.tile_pool(name="psB", bufs=3, space="PSUM"))

        # constants
        w1_sb = cpool.tile([EDGE_DIM, HIDDEN], dt.float32r)
        nc.sync.dma_start(w1_sb[:], w1_d[:])
        b1_sb = cpool.tile([HIDDEN, 1], dt.float32)
        nc.sync.dma_start(b1_sb[:], b1_d[:])
        w2_sb = cpool.tile([HIDDEN, RW], dt.float32r)
        nc.sync.dma_start(w2_sb[:], w2_d[:])
        b2_sb = cpool.tile([1, RW], dt.float32r)
        nc.sync.dma_start(b2_sb[:], b2_d[:])
        ones_sb = cpool.tile([1, 128], dt.float32r)
        nc.sync.dma_start(ones_sb[:], ones_d[:])
        uw_sb = cpool.tile([128, ESP // 16], dt.int16)
        nc.sync.dma_start(uw_sb[:], uw_d[:])

        def body():
            _body(nc, tc, dt, cpool, inpool, hpool, wpool, opool, pps, ppr,
                  efT_d, basis_d, f_d, uw_sb, w1_sb, b1_sb, w2_sb, b2_sb,
                  ones_sb, out_d)

        if reps == 1:
            body()
        else:
            with tc.For_i(0, reps, 1):
                body()

    nc.compile()
    return nc


def _body(nc, tc, dt, cpool, inpool, hpool, wpool, opool, pps, ppr,
          efT_d, basis_d, f_d, uw_sb, w1_sb, b1_sb, w2_sb, b2_sb,
          ones_sb, out_d):
    mdt = mybir.dt
    if True:
        for b in range(NBLK):
            # block loads
            efT_sb = inpool.tile([EDGE_DIM, EBLK], dt.float32r, tag="efT")
            nc.sync.dma_start(efT_sb[:], efT_d[:, b * EBLK:(b + 1) * EBLK])
            basis_sb = inpool.tile([128, BLK * 27], dt.float32, tag="basis")
            nc.sync.dma_start(basis_sb[:], basis_d[:, b * BLK * 27:(b + 1) * BLK * 27])
            fe_sb = inpool.tile([128, BLK, FPAD], dt.float32, tag="fe")
            for g in range(EBLK // 1024):
                i0 = b * (EBLK // 16) + g * 64
                nc.gpsimd.dma_gather(
                    fe_sb[:, g * 8:(g + 1) * 8, :], f_d[:],
                    uw_sb[:, i0:i0 + 64],
                    num_idxs=1024, num_idxs_reg=1024, elem_size=FPAD,
                )

            # h.T = relu(W1.T @ efT + b1): [128h, EBLK]
            hT_sb = hpool.tile([HIDDEN, EBLK], dt.float32r, tag="hT")
            for q in range(EBLK // 512 if "nomlp" not in ABL else 0):
                hT_ps = pps.tile([HIDDEN, 512], dt.float32, tag="hTps")
                nc.tensor.matmul(hT_ps[:], w1_sb[:],
                                 efT_sb[:, q * 512:(q + 1) * 512],
                                 start=True, stop=True)
                nc.scalar.activation(hT_sb[:, q * 512:(q + 1) * 512], hT_ps[:],
                                     mybir.ActivationFunctionType.Relu,
                                     bias=b1_sb[:], scale=1.0)

            out_sb = opool.tile([128, BLK, 48], dt.float32, tag="outsb")

            for t in range(BLK):
                # rw = hT_chunk.T @ W2 + b2 : [128e, 768] in PSUM
                rw_ps = ppr.tile([128, RW], dt.float32, tag="rwps")
                hT_c = hT_sb[:, t * 128:(t + 1) * 128]
                if "nomlp" not in ABL:
                    nc.tensor.matmul(rw_ps[:, 0:512], hT_c, w2_sb[:, 0:512],
                                     start=True, stop=False)
                    nc.tensor.matmul(rw_ps[:, 0:512], ones_sb[:], b2_sb[:, 0:512],
                                     start=False, stop=True)
                    nc.tensor.matmul(rw_ps[:, 512:RW], hT_c, w2_sb[:, 512:RW],
                                     start=True, stop=False)
                    nc.tensor.matmul(rw_ps[:, 512:RW], ones_sb[:], b2_sb[:, 512:RW],
                                     start=False, stop=True)

                # tmp[e, m, k] = sum_d fe[e, m, d] * basis[e, d, k]
                fe_t = fe_sb[:, t, 0:48].rearrange("p (m d) -> p m d", m=M1, d=D1)
                ba_t = basis_sb[:, t * 27:(t + 1) * 27].rearrange(
                    "p (d k) -> p d k", d=D1, k=NREPS * D2)
                tmp_sb = wpool.tile([128, M1, NREPS * D2], dt.float32, tag="tmp")
                if "notmp" not in ABL:
                    # one fused mult over (m,k,d) then reduce innermost d;
                    # alternate tiles go to GPSIMD (otherwise idle) as
                    # mult + two strided adds (no X-reduce on POOL).
                    K9 = NREPS * D2
                    tpd = wpool.tile([128, M1, K9, D1], dt.float32, tag="tprodd")
                    fe_b = fe_t.unsqueeze(2).broadcast_to([128, M1, K9, D1])  # [p,m,k*,d]
                    ba_kd = basis_sb[:, t * 27:(t + 1) * 27].rearrange(
                        "p (d k) -> p k d", d=D1, k=K9)
                    ba_b = ba_kd.unsqueeze(1).broadcast_to([128, M1, K9, D1])  # [p,m*,k,d]
                    nc.vector.tensor_tensor(tpd[:], fe_b, ba_b, mybir.AluOpType.mult)
                    nc.vector.tensor_reduce(tmp_sb[:], tpd[:],
                                            axis=mybir.AxisListType.X,
                                            op=mybir.AluOpType.add)



                # out[e, i, dd] = sum_{m,r} rw[e, i, m, r] * tmp[e, m, r, dd]
                # fused: one mult into (i, dd, m, r) layout, one XY-reduce
                if "nostep5" not in ABL:
                    rw_b = rw_ps[:].rearrange(
                        "p (i m r) -> p i m r", i=M2, m=M1, r=NREPS
                    ).unsqueeze(2).broadcast_to([128, M2, D2, M1, NREPS])
                    tmp_b = tmp_sb[:].rearrange(
                        "p m (r dd) -> p dd m r", r=NREPS, dd=D2
                    ).unsqueeze(1).broadcast_to([128, M2, D2, M1, NREPS])
                    prod = wpool.tile([128, M2 * D2, M1, NREPS], dt.float32, tag="prod")
                    nc.vector.tensor_tensor(
                        prod[:].rearrange("p (i dd) m r -> p i dd m r", i=M2, dd=D2),
                        rw_b, tmp_b, mybir.AluOpType.mult)
                    nc.vector.tensor_reduce(out_sb[:, t, :], prod[:],
                                            axis=mybir.AxisListType.XY,
                                            op=mybir.AluOpType.add)

            if "nostep5" not in ABL:
                nc.sync.dma_start(out_d[:, b * BLK * 48:(b + 1) * BLK * 48],
                                  out_sb[:].rearrange("p b k -> p (b k)"))


def _get_nc(reps=1):
    key = ("nc", reps)
    if key not in _CACHE:
        _CACHE[key] = _build(reps)
    return _CACHE[key]


def _prep_core(U_c, basis_c, ef_c, f, W1, b1, W2, b2):
    """Build one core's input map (host-side layout/swizzle)."""
    npad = ESP - U_c.shape[0]
    U_p = np.concatenate([np.asarray(U_c, np.int64), np.zeros(npad, np.int64)])
    basis_p = np.concatenate(
        [np.asarray(basis_c, np.float32).reshape(-1, 27),
         np.zeros((npad, 27), np.float32)], axis=0)
    ef_p = np.concatenate(
        [np.asarray(ef_c, np.float32),
         np.zeros((npad, EDGE_DIM), np.float32)], axis=0)

    efT = np.ascontiguousarray(ef_p.T)                                   # [32, ESP]
    basisp = np.ascontiguousarray(
        basis_p.reshape(NTILES, 128, 27).transpose(1, 0, 2).reshape(128, NTILES * 27))
    # wrapped per 1024-index gather group: uw[p, c] = U[(c//64)*1024 + (c%64)*16 + p]
    uw16 = U_p.astype(np.int16).reshape(ESP // 1024, 64, 16).transpose(2, 0, 1)
    uw = np.ascontiguousarray(
        np.tile(uw16.reshape(16, ESP // 16), (8, 1)))  # [128, ESP//16], 8x for Q7 cores
    fpad = np.zeros((N, FPAD), np.float32)
    fpad[:, :M1 * D1] = np.asarray(f, np.float32).reshape(N, M1 * D1)
    return {
        "efT": efT,
        "basisp": basisp,
        "uw": uw,
        "fpad": fpad,
        "w1": np.asarray(W1, np.float32),
        "b1": np.asarray(b1, np.float32).reshape(HIDDEN, 1),
        "w2": np.asarray(W2, np.float32),
        "b2": np.asarray(b2, np.float32).reshape(1, RW),
        "ones1": np.ones((1, 128), np.float32),
    }


def kernel(U, basis, edge_feats, f, W1, b1, W2, b2):
    U = np.asarray(U)
    basis = np.asarray(basis, np.float32)
    edge_feats = np.asarray(edge_feats, np.float32)
    nc = _get_nc()
    in_maps = []
    for c in range(NCORES):
        sl = slice(c * ES, (c + 1) * ES)
        in_maps.append(_prep_core(U[sl], basis[sl], edge_feats[sl],
                                  f, W1, b1, W2, b2))
    res = run_bass_kernel_spmd(nc, in_maps, core_ids=list(range(NCORES)))
    outs = []
    for c in range(NCORES):
        op = res.results[c]["outp"]                                   # [128, NTILES*48]
        o = op.reshape(128, NTILES, 48).transpose(1, 0, 2).reshape(ESP, 48)
        outs.append(o[:ES])
    return np.concatenate(outs, axis=0).reshape(E, M2, D2).astype(np.float32)


if __name__ == "__main__":
    # quick self-run with random data
    rng = np.random.default_rng(0)
    inputs = {
        "U": rng.integers(0, N, size=E),
        "basis": rng.standard_normal((E, D1, NREPS * D2), dtype=np.float32),
        "edge_feats": rng.standard_normal((E, EDGE_DIM), dtype=np.float32),
        "f": rng.standard_normal((N, M1, D1), dtype=np.float32),
        "W1": (rng.standard_normal((EDGE_DIM, HIDDEN), dtype=np.float32) / np.sqrt(EDGE_DIM)),
        "b1": rng.standard_normal(HIDDEN, dtype=np.float32) * 0.02,
        "W2": (rng.standard_normal((HIDDEN, RW), dtype=np.float32) / np.sqrt(HIDDEN)),
        "b2": rng.standard_normal(RW, dtype=np.float32) * 0.02,
    }
    out = kernel(**inputs)
    print(out.shape, out.dtype)
